# revision 87
# baseline (speedup 1.0000x reference)
"""DeepseekV3 MoE gate (moe_routing) for 8x TRN2 NeuronCores.

Sharding: data-parallel over tokens. Each core gets a 2048-token shard of x;
the small gate weight [7168, 256] and bias are replicated.

Default mode "f16dr": fp16 main matmul + ONE DoubleRow fp8e4 matmul carrying
BOTH precision-correction terms on a stacked 256-deep contraction:

  logits = xh16 @ W16 + 2^-17 * (xl8 @ W8 + xh8 @ dW8)
    xh16 = fp16(x)                  streamed 2B/elem
    xl8  = e4m3((x - xh16)*2^11)    streamed 1B/elem
    xh8  = e4m3(xh16)               cast on-chip (ACT)
    W16  = fp16(W) resident; W8 = e4m3(W16*2^6) cast on-chip
    dW8  = e4m3((W - W16)*2^17)     streamed once

Per (k-chunk, 128-token block): fp16 matmul (256 cy) into the main half of a
shared PSUM bank + DoubleRow fp8 matmul (128 cy) into the corr half -> 384
cy/k-chunk vs 768 for the f16x3 split, at ~1e-5 logit rms error (ranking
fidelity within a hair of f16x3: ~1 near-tie set flip in 16K tokens).

x streams are block-paneled host-side ([P, n_blk, n_k, blk]) so every DMA
descriptor is >=1KB; 16 flat 128-token blocks, 8 PSUM banks in flight; W
loads ride the same SP/HWDGE queue just-in-time during block 0; the corr
matmuls lag one x-tile behind the mains so the ACT cast pipeline never
stalls the PE; routing (sigmoid + grouped top-k) is emitted one block behind
the k-loops and recovers the selected sigmoid scores in rank order via a
second Max8/MaxIndex plus an 8x8 index-match (no gather engine needed).
"""

import sys

if "/opt/trn_rl_repo" not in sys.path:
    sys.path.insert(0, "/opt/trn_rl_repo")

from contextlib import ExitStack

import ml_dtypes
import numpy as np

import concourse.bass as bass
import concourse.mybir as mybir
import concourse.tile as tile
from concourse import bacc
from concourse._compat import with_exitstack

H = 7168
E = 256
G = 8
EPG = E // G  # 32
K = 8
NEG = -1.0e30
ROUTE_SCALE = 2.5
P = 128

N_CORES = 8
T_FULL = 16384
T_CORE = T_FULL // N_CORES  # 2048

MODE = "f16dr"  # "f32r" | "f32" | "f16x3" | "f16dr"
BLK = 512
KPACK_DEFAULT = 4
XBUFS = 4

DR_BLK = 128
DR_KPACK = 14


def np_algo_reference(x, W, bias):
    """Numpy mirror of the kernel algorithm (for validation in tests)."""
    x = x.astype(np.float32)
    T = x.shape[0]
    logits = (x.astype(np.float64) @ W.astype(np.float64)).astype(np.float32)
    s0 = (1.0 / (1.0 + np.exp(-logits.astype(np.float64)))).astype(np.float32)
    b = s0 + bias.astype(np.float32)
    bg = b.reshape(T, G, EPG)
    top2 = np.sort(bg, axis=-1)[:, :, -2:]
    gs = (top2[:, :, 0] + top2[:, :, 1]).astype(np.float32)
    gsort = np.sort(gs, axis=-1)[:, ::-1]
    thresh = gsort[:, 3:4]
    pen = np.where(gs >= thresh, np.float32(0.0), np.float32(NEG))
    ms = b + np.repeat(pen, EPG, axis=1)
    order = np.argsort(-ms, axis=-1, kind="stable")[:, :K]
    s0sel = np.take_along_axis(s0, order, axis=-1)
    q = np.arange(K, 0, -1).astype(np.float32)
    z = (q[None, :] + s0sel).astype(np.float32)
    s0sel_rt = (z - q[None, :]).astype(np.float32)
    ssum = s0sel_rt.sum(-1, keepdims=True, dtype=np.float32)
    wts = (s0sel_rt * ((np.float32(1.0) / ssum) * np.float32(ROUTE_SCALE))).astype(
        np.float32
    )
    return wts, order.astype(np.int32)


@with_exitstack
def _gate_kernel(
    ctx: ExitStack,
    tc: tile.TileContext,
    outs,
    ins,
    T_core: int,
    BLK: int = 512,
    mode: str = "f32r",
    repeat: int = 1,
    taper: bool = False,
):
    nc = tc.nc
    wts_d, sel_d = outs
    if mode == "f16x3":
        xh_d, xl_d, wh_d, wl_d, bias_d = ins
    else:
        xT_d, w_d, bias_d = ins

    n_k = H // P  # 56
    KPACK = KPACK_DEFAULT  # k-chunks per x DMA (fewer, larger DMAs)
    assert n_k % KPACK == 0
    n_tiles = T_core // P

    # Uniform block schedule (HW-verified configuration). With taper=True the
    # final blocks shrink (512->256->128->128) so the post-matmul routing
    # tail drains one tile deep instead of four.
    if taper:
        blocks = []
        t = 0
        rem = T_core
        while rem > 0:
            if rem > BLK:
                bs = BLK
            elif rem == BLK and BLK >= 4 * P:
                bs = BLK // 2
            elif rem > 2 * P:
                bs = rem - 2 * P
            else:
                bs = P
            bs = min(bs, rem)
            blocks.append((t, bs))
            t += bs
            rem -= bs
    else:
        blocks = [(i * BLK, BLK) for i in range(T_core // BLK)]

    f32 = mybir.dt.float32
    f16 = mybir.dt.float16
    assert T_core % BLK == 0 and BLK % P == 0

    const = ctx.enter_context(tc.tile_pool(name="const", bufs=1))
    wpool = ctx.enter_context(tc.tile_pool(name="wpool", bufs=1))
    xpool = ctx.enter_context(tc.tile_pool(name="xpool", bufs=XBUFS))
    ppool = ctx.enter_context(tc.tile_pool(name="ppool", bufs=8, space="PSUM"))
    spool = ctx.enter_context(tc.tile_pool(name="spool", bufs=3))
    opool = ctx.enter_context(tc.tile_pool(name="opool", bufs=3))

    # ---- constants ----
    bias_bc = const.tile([P, E], f32)
    nc.sync.dma_start(bias_bc[:], bias_d.unsqueeze(0).to_broadcast([P, E]))

    qrow32 = const.tile([P, K], f32)
    for k in range(K):
        nc.vector.memset(qrow32[:, k : k + 1], float(K - k))

    # output accumulators: one SBUF row-block per 128-token tile, DMA'd once
    outw_acc = const.tile([P, n_tiles, K], f32)
    outs_acc = const.tile([P, n_tiles, K], mybir.dt.int32)

    # ---- resident weights ----
    if mode == "f16x3":
        w_all_h = wpool.tile([P, n_k, E], f16)
        w_all_l = wpool.tile([P, n_k, E], f16)
        wh_view = wh_d.rearrange("(k p) e -> p k e", p=P)
        wl_view = wl_d.rearrange("(k p) e -> p k e", p=P)
        # SWDGE path keeps the HWDGE ring free for the x stream
        wsplit = 14
        for k0 in range(0, n_k, wsplit):
            k1 = min(k0 + wsplit, n_k)
            nc.gpsimd.dma_start(w_all_h[:, k0:k1, :], wh_view[:, k0:k1, :])
            nc.gpsimd.dma_start(w_all_l[:, k0:k1, :], wl_view[:, k0:k1, :])
    else:
        mdt = mybir.dt.float32r if mode == "f32r" else f32
        w_all = wpool.tile([P, n_k, E], mdt)
        w_view = w_d.rearrange("(k p) e -> p k e", p=P)
        wsplit = 14  # k-chunks per W DMA: let early chunks land first
        for k0 in range(0, n_k, wsplit):
            k1 = min(k0 + wsplit, n_k)
            nc.gpsimd.dma_start(w_all[:, k0:k1, :], w_view[:, k0:k1, :])

    # ---- main loop ----
    for rep, (tb, (t0, bs)) in [
        (r, b) for r in range(repeat) for b in enumerate(blocks)
    ]:
        n_sub = bs // P
        psums = []
        for s in range(n_sub):
            pt = ppool.tile([P, E], f32, name=f"psum_{rep}_{tb}_{s}", tag="psum")
            psums.append(pt)

        for k0 in range(0, n_k, KPACK):
            if mode == "f16x3":
                xch = xpool.tile([P, KPACK, bs], f16, tag="xch")
                xcl = xpool.tile([P, KPACK, bs], f16, tag="xcl")
                nc.sync.dma_start(
                    xch[:],
                    xh_d[k0 * P : (k0 + KPACK) * P, t0 : t0 + bs].rearrange(
                        "(kk p) t -> p kk t", p=P
                    ),
                )
                nc.sync.dma_start(
                    xcl[:],
                    xl_d[k0 * P : (k0 + KPACK) * P, t0 : t0 + bs].rearrange(
                        "(kk p) t -> p kk t", p=P
                    ),
                )
                for kk in range(KPACK):
                    k = k0 + kk
                    start = k == 0
                    stop = k == n_k - 1
                    for s in range(n_sub):
                        lh = xch[:, kk, s * P : (s + 1) * P]
                        ll = xcl[:, kk, s * P : (s + 1) * P]
                        # xh stationary twice in a row -> cheaper weight reload
                        nc.tensor.matmul(
                            psums[s][:], lh, w_all_h[:, k, :], start=start, stop=False
                        )
                        nc.tensor.matmul(
                            psums[s][:], lh, w_all_l[:, k, :], start=False, stop=False
                        )
                        nc.tensor.matmul(
                            psums[s][:], ll, w_all_h[:, k, :], start=False, stop=stop
                        )
            else:
                xc = xpool.tile([P, KPACK, bs], mdt, tag="xch")
                nc.sync.dma_start(
                    xc[:],
                    xT_d[k0 * P : (k0 + KPACK) * P, t0 : t0 + bs].rearrange(
                        "(kk p) t -> p kk t", p=P
                    ),
                )
                for kk in range(KPACK):
                    k = k0 + kk
                    for s in range(n_sub):
                        nc.tensor.matmul(
                            psums[s][:],
                            xc[:, kk, s * P : (s + 1) * P],
                            w_all[:, k, :],
                            start=(k == 0),
                            stop=(k == n_k - 1),
                        )

        for s in range(n_sub):
            trow = t0 + s * P
            s0 = spool.tile([P, E], f32, tag="s0")
            nc.scalar.activation(
                s0[:], psums[s][:], mybir.ActivationFunctionType.Sigmoid
            )
            b = spool.tile([P, E], f32, tag="b")
            nc.vector.tensor_add(b[:], s0[:], bias_bc[:])
            gmax = opool.tile([P, G * 8], f32, tag="gmax")
            for g in range(G):
                nc.vector.max(
                    out=gmax[:, g * 8 : (g + 1) * 8],
                    in_=b[:, g * EPG : (g + 1) * EPG],
                )
            gv = gmax[:].rearrange("p (g c) -> p g c", g=G)
            gs = opool.tile([P, G], f32, tag="gs")
            nc.vector.tensor_add(gs[:], gv[:, :, 0], gv[:, :, 1])
            gtop = opool.tile([P, 8], f32, tag="gtop")
            nc.vector.max(out=gtop[:], in_=gs[:])
            pen = opool.tile([P, G], f32, tag="pen")
            nc.vector.tensor_scalar(
                pen[:],
                gs[:],
                gtop[:, 3:4],
                None,
                op0=mybir.AluOpType.is_ge,
            )
            nc.vector.tensor_scalar(
                pen[:],
                pen[:],
                1.0,
                -NEG,
                op0=mybir.AluOpType.subtract,
                op1=mybir.AluOpType.mult,
            )
            ms = spool.tile([P, E], f32, tag="ms")
            pen_bc = pen[:].unsqueeze(2).to_broadcast([P, G, EPG])
            nc.vector.tensor_add(
                ms[:].rearrange("p (g c) -> p g c", g=G),
                b[:].rearrange("p (g c) -> p g c", g=G),
                pen_bc,
            )
            vals8 = opool.tile([P, K], f32, tag="vals8")
            nc.vector.max(out=vals8[:], in_=ms[:])
            idxu = opool.tile([P, K], mybir.dt.uint16, tag="idxu")
            nc.vector.max_index(idxu[:], vals8[:], ms[:])
            # z[p,e] = s0[p,e] + #{k : ms[p,e] >= vals8[p,k]}
            # selected rank-r expert lands in band (8-r, 9-r); others in (0,1)
            z = spool.tile([P, E], f32, tag="z")
            nc.vector.scalar_tensor_tensor(
                z[:],
                ms[:],
                vals8[:, 0:1],
                s0[:],
                op0=mybir.AluOpType.is_ge,
                op1=mybir.AluOpType.add,
            )
            for k in range(1, K):
                nc.vector.scalar_tensor_tensor(
                    z[:],
                    ms[:],
                    vals8[:, k : k + 1],
                    z[:],
                    op0=mybir.AluOpType.is_ge,
                    op1=mybir.AluOpType.add,
                )
            zv = opool.tile([P, K], f32, tag="zv")
            nc.vector.max(out=zv[:], in_=z[:])
            s0sel = opool.tile([P, K], f32, tag="s0sel")
            nc.vector.tensor_sub(s0sel[:], zv[:], qrow32[:])
            ssum = opool.tile([P, 1], f32, tag="ssum")
            nc.vector.tensor_reduce(
                ssum[:], s0sel[:], axis=mybir.AxisListType.X, op=mybir.AluOpType.add
            )
            rec = opool.tile([P, 1], f32, tag="rec")
            nc.vector.reciprocal(rec[:], ssum[:])
            ti = trow // P
            nc.vector.tensor_scalar(
                outw_acc[:, ti, :],
                s0sel[:],
                rec[:],
                ROUTE_SCALE,
                op0=mybir.AluOpType.mult,
                op1=mybir.AluOpType.mult,
            )
            nc.vector.tensor_copy(outs_acc[:, ti, :], idxu[:])

        # flush this block's outputs so only the last block's tail is exposed
        ti0 = t0 // P
        nc.sync.dma_start(
            wts_d[t0 : t0 + bs, :].rearrange("(tt p) k -> p tt k", p=P),
            outw_acc[:, ti0 : ti0 + n_sub, :],
        )
        nc.sync.dma_start(
            sel_d[t0 : t0 + bs, :].rearrange("(tt p) k -> p tt k", p=P),
            outs_acc[:, ti0 : ti0 + n_sub, :],
        )


@with_exitstack
def _gate_kernel_f16dr(
    ctx: ExitStack,
    tc: tile.TileContext,
    outs,
    ins,
    T_core: int,
    BLK: int = 512,
    kpack: int = 4,
    xbufs: int = 8,
):
    """fp16 main term + one DoubleRow fp8e4 matmul for both correction terms.

    logits = xh16 @ W16 + 2^-17 * (xl8 @ W8 + xh8 @ dW8)
      xh16 = fp16(x)                 (streamed, 2B/elem)
      xl8  = e4m3((x - xh16)*2^11)   (streamed, 1B/elem)
      xh8  = e4m3(xh16)              (cast on ACT engine on-chip)
      W16  = fp16(W)                 (resident)
      W8   = e4m3(W*2^6), dW8 = e4m3((W - W16)*2^17)   (resident, packed)

    Per (k-chunk, 128-token subtile): one fp16 matmul (256 cy) into the main
    half of a shared PSUM bank + one DoubleRow fp8 matmul (128 cy, contraction
    [xl8; xh8] vs [W8; dW8]) into the corr half. DVE combines the two halves
    with the 2^-17 scale, ACT applies sigmoid, and the DVE routing tail is
    identical to the f16x3 mode.

    PSUM: each [128, 2, 256] tile is one bank holding (main, corr) for one
    subtile; 4 banks per 512-token block, 8 banks -> 2 blocks in flight. The
    main k=0 matmul (start=True) clears the bank's has_written bits; the corr
    k=0 matmul runs start=False and lands on pending-zero bytes, which the HW
    treats as overwrite -- so the main k=0 matmul MUST issue first (guaranteed
    by emission-order priorities plus the tiny first W16 DMA chunk).
    """
    nc = tc.nc
    wts_d, sel_d = outs
    xh_d, xl8_d, w16_d, dw8_d, bias_d = ins

    n_k = H // P  # 56
    n_tiles = T_core // P
    n_blk = T_core // BLK
    assert T_core % BLK == 0 and n_k % kpack == 0

    f32 = mybir.dt.float32
    f16 = mybir.dt.float16
    f8 = mybir.dt.float8e4
    CORR_SCALE = float(2.0**-17)

    const = ctx.enter_context(tc.tile_pool(name="const", bufs=1))
    wpool = ctx.enter_context(tc.tile_pool(name="wpool", bufs=1))
    xpool = ctx.enter_context(tc.tile_pool(name="xpool", bufs=xbufs))
    upool = ctx.enter_context(tc.tile_pool(name="upool", bufs=xbufs))
    ppool = ctx.enter_context(tc.tile_pool(name="ppool", bufs=8, space="PSUM"))
    spool = ctx.enter_context(tc.tile_pool(name="spool", bufs=3))
    opool = ctx.enter_context(tc.tile_pool(name="opool", bufs=3))

    # ---- resident weights ----
    # W DMAs ride the same SP/HWDGE queue as the x stream, emitted just
    # before the block-0 x tile that first needs each k-range: the SP stream
    # order guarantees W(k) beats the x tiles of later k-ranges, and the
    # one-time block-0 DMA deficit is repaid by DMA idle in later blocks.
    w16_all = wpool.tile([P, n_k, E], f16)
    # vp pair-dim OUTER so the dW8 half is a contiguous DMA target and the
    # W8 half is an ACT cast target (derived on-chip from w16: saves 1.8MB
    # of front-loaded DMA)
    vp_all = wpool.tile([P, 2, n_k, E], f8)
    w16_view = w16_d.rearrange("(k p) e -> p k e", p=P)
    dw8_view = dw8_d  # host-paneled [P, n_k, E]

    # ---- constants ----
    # bias rides the Pool/SWDGE queue (behind the first w16 chunk) so it
    # never delays the x stream
    bias_bc = const.tile([P, E], f32)

    qrow32 = const.tile([P, K], f32)
    for k in range(K):
        nc.vector.memset(qrow32[:, k : k + 1], float(K - k))

    outw_acc = const.tile([P, n_tiles, K], f32)
    outs_acc = const.tile([P, n_tiles, K], mybir.dt.int32)

    # ---- main loop ----
    # x streams are block-paneled host-side: [P, n_blk, n_k, BLK], so one DMA
    # descriptor covers kpack k-chunks contiguously per partition (>=1KB even
    # at BLK=256). Flat BLK=256 blocks: 2 PSUM banks each -> 4 blocks in
    # flight, and only ~2 subtiles of routing are exposed past the last
    # matmul.
    n_sub = BLK // P

    # The corr matmuls lag one x-tile behind the mains in the (in-order)
    # PE stream so the ACT cast they depend on has a full tile of main
    # matmul time to complete. `corr_lag` threads that state across
    # emission units (and across blocks in the prologue).
    corr_lag = [None]

    def emit_corr(u_t, k0, klen, psums):
        for kk in range(klen):
            k = k0 + kk
            for s in range(n_sub):
                nc.tensor.matmul(
                    psums[s][:, 1, :],
                    u_t[:, :, kk, s * P : (s + 1) * P],
                    vp_all[:, :, k, :],
                    start=False,
                    stop=(k == n_k - 1),
                    perf_mode=mybir.MatmulPerfMode.DoubleRow,
                    skip_group_check=True,
                )

    def emit_unit(tb, k0, klen, psums):
        """x DMAs + cast + main matmuls for one (block, k-range) unit."""
        xh_t = xpool.tile(
            [P, kpack, BLK], f16, tag="xch", name=f"xh_{tb}_{k0}"
        )
        nc.sync.dma_start(xh_t[:, 0:klen, :], xh_d[:, tb, k0 : k0 + klen, :])
        u_t = upool.tile([P, 2, kpack, BLK], f8, tag="uch", name=f"u_{tb}_{k0}")
        nc.sync.dma_start(
            u_t[:, 0, 0:klen, :], xl8_d[:, tb, k0 : k0 + klen, :]
        )
        # fp16 -> e4m3 cast of the hi part (ACT engine), one instruction
        # for the whole [P, klen, BLK] slab
        nc.scalar.copy(u_t[:, 1, 0:klen, :], xh_t[:, 0:klen, :])
        for kk in range(klen):
            k = k0 + kk
            for s in range(n_sub):
                nc.tensor.matmul(
                    psums[s][:, 0, :],
                    xh_t[:, kk, s * P : (s + 1) * P],
                    w16_all[:, k, :],
                    start=(k == 0),
                    stop=(k == n_k - 1),
                )
        if corr_lag[0] is not None:
            emit_corr(*corr_lag[0])
        corr_lag[0] = (u_t, k0, klen, psums)

    # Split the first ranges so the first matmul isn't gated on a
    # full-range W + x DMA.
    pro_ranges = [(0, 4), (4, kpack - 4)] + [
        (k0, kpack) for k0 in range(kpack, n_k, kpack)
    ]
    std_ranges = [(k0, kpack) for k0 in range(0, n_k, kpack)]

    def emit_prologue(pro_blocks, psums_by_tb):
        """k-outer sweep over the first blocks: the W stream (w16 DMA +
        dW8 DMA + on-chip W8 cast, emitted just-in-time per k-range) is
        consumed across `len(pro_blocks)` blocks of PE work instead of one,
        shrinking the one-time W-vs-x DMA deficit."""
        for ri, (k0, klen) in enumerate(pro_ranges):
            # first chunk via SWDGE: Pool's desc-gen path reaches first-byte
            # ~0.2us sooner than SP/HWDGE at kernel start
            wq = nc.gpsimd if ri == 0 else nc.sync
            wq.dma_start(
                w16_all[:, k0 : k0 + klen, :],
                w16_view[:, k0 : k0 + klen, :],
            )
            if ri == 0:
                nc.gpsimd.dma_start(
                    bias_bc[:], bias_d.unsqueeze(0).to_broadcast([P, E])
                )
            for i, tb in enumerate(pro_blocks):
                emit_unit(tb, k0, klen, psums_by_tb[tb])
                if i == 0:
                    # dW8 after the first block's xl8; W8 cast from w16
                    nc.sync.dma_start(
                        vp_all[:, 1, k0 : k0 + klen, :],
                        dw8_view[:, k0 : k0 + klen, :],
                    )
                    nc.scalar.activation(
                        vp_all[:, 0, k0 : k0 + klen, :],
                        w16_all[:, k0 : k0 + klen, :],
                        mybir.ActivationFunctionType.Copy,
                        scale=float(2.0**6),
                    )

    def emit_kloop(tb, psums, routing_pending=None):
        for i, (k0, klen) in enumerate(std_ranges):
            emit_unit(tb, k0, klen, psums)
            if i == 0 and routing_pending:
                # Emit the previous block's routing COMPUTE right after this
                # block's first tile: its ACT ops (ct/sigmoid) land behind
                # only one cast in the ACT FIFO instead of the whole k-loop's
                # casts, so the chain starts as soon as its psums complete.
                # (Out-DMAs stay deferred: they would head-of-line block the
                # SP/ACT queues' x-stream DMAs.)
                for args in routing_pending:
                    emit_routing(*args)

    def emit_routing(t0, psums):
        for s in range(n_sub):
            trow = t0 + s * P
            # DVE may read only ONE PSUM operand per instruction: ACT
            # scale-copies the corr half to SBUF, DVE adds the main half.
            ct = spool.tile([P, E], f32, tag="ct")
            nc.scalar.activation(
                ct[:],
                psums[s][:, 1, :],
                mybir.ActivationFunctionType.Copy,
                scale=CORR_SCALE,
            )
            lg = spool.tile([P, E], f32, tag="lg")
            nc.vector.tensor_add(lg[:], ct[:], psums[s][:, 0, :])
            s0 = spool.tile([P, E], f32, tag="s0")
            nc.scalar.activation(
                s0[:], lg[:], mybir.ActivationFunctionType.Sigmoid
            )
            b = spool.tile([P, E], f32, tag="b")
            nc.vector.tensor_add(b[:], s0[:], bias_bc[:])
            gmax = opool.tile([P, G * 8], f32, tag="gmax")
            for g in range(G):
                nc.vector.max(
                    out=gmax[:, g * 8 : (g + 1) * 8],
                    in_=b[:, g * EPG : (g + 1) * EPG],
                )
            gv = gmax[:].rearrange("p (g c) -> p g c", g=G)
            gs = opool.tile([P, G], f32, tag="gs")
            nc.vector.tensor_add(gs[:], gv[:, :, 0], gv[:, :, 1])
            gtop = opool.tile([P, 8], f32, tag="gtop")
            nc.vector.max(out=gtop[:], in_=gs[:])
            pen = opool.tile([P, G], f32, tag="pen")
            nc.vector.tensor_scalar(
                pen[:],
                gs[:],
                gtop[:, 3:4],
                NEG,
                op0=mybir.AluOpType.is_lt,
                op1=mybir.AluOpType.mult,
            )
            ms = spool.tile([P, E], f32, tag="ms")
            pen_bc = pen[:].unsqueeze(2).to_broadcast([P, G, EPG])
            nc.vector.tensor_add(
                ms[:].rearrange("p (g c) -> p g c", g=G),
                b[:].rearrange("p (g c) -> p g c", g=G),
                pen_bc,
            )
            vals8 = opool.tile([P, K], f32, tag="vals8")
            nc.vector.max(out=vals8[:], in_=ms[:])
            idxu = opool.tile([P, K], mybir.dt.uint16, tag="idxu")
            nc.vector.max_index(idxu[:], vals8[:], ms[:])
            # Recover selected s0 values in ms-rank order WITHOUT the 8-pass
            # z-band chain: z = s0 + (ms >= vals8[7]) puts the selected set in
            # band (1,2); Max8/MaxIndex over z gives that set in s0-order, and
            # an 8x8 index-match permutes it into rank order (exact in fp32).
            z = spool.tile([P, E], f32, tag="z")
            nc.vector.scalar_tensor_tensor(
                z[:],
                ms[:],
                vals8[:, 7:8],
                s0[:],
                op0=mybir.AluOpType.is_ge,
                op1=mybir.AluOpType.add,
            )
            zq = opool.tile([P, K], f32, tag="zq")
            nc.vector.max(out=zq[:], in_=z[:])
            idxu2 = opool.tile([P, K], mybir.dt.uint16, tag="idxu2")
            nc.vector.max_index(idxu2[:], zq[:], z[:])
            idxuf = opool.tile([P, K], f32, tag="idxuf")
            nc.vector.tensor_copy(idxuf[:], idxu[:])
            idxu2f = opool.tile([P, K], f32, tag="idxu2f")
            nc.vector.tensor_copy(idxu2f[:], idxu2[:])
            eqm = opool.tile([P, K, K], f32, tag="eqm")
            nc.vector.tensor_tensor(
                eqm[:],
                idxuf[:].unsqueeze(2).to_broadcast([P, K, K]),
                idxu2f[:].unsqueeze(1).to_broadcast([P, K, K]),
                op=mybir.AluOpType.is_equal,
            )
            nc.vector.scalar_tensor_tensor(
                eqm[:],
                zq[:].unsqueeze(1).to_broadcast([P, K, K]),
                -1.0,
                eqm[:],
                op0=mybir.AluOpType.add,
                op1=mybir.AluOpType.mult,
            )
            s0sel = opool.tile([P, K], f32, tag="s0sel")
            nc.vector.tensor_reduce(
                s0sel[:].unsqueeze(2),
                eqm[:],
                axis=mybir.AxisListType.X,
                op=mybir.AluOpType.add,
            )
            ssum = opool.tile([P, 1], f32, tag="ssum")
            nc.vector.tensor_reduce(
                ssum[:], s0sel[:], axis=mybir.AxisListType.X, op=mybir.AluOpType.add
            )
            rec = opool.tile([P, 1], f32, tag="rec")
            nc.vector.reciprocal(rec[:], ssum[:])
            ti = trow // P
            nc.vector.tensor_scalar(
                outw_acc[:, ti, :],
                s0sel[:],
                rec[:],
                ROUTE_SCALE,
                op0=mybir.AluOpType.mult,
                op1=mybir.AluOpType.mult,
            )
            nc.vector.tensor_copy(outs_acc[:, ti, :], idxu[:])

    def emit_routing_dma(t0, psums):
        ti0 = t0 // P
        nc.sync.dma_start(
            wts_d[t0 : t0 + BLK, :].rearrange("(tt p) k -> p tt k", p=P),
            outw_acc[:, ti0 : ti0 + n_sub, :],
        )
        nc.scalar.dma_start(
            sel_d[t0 : t0 + BLK, :].rearrange("(tt p) k -> p tt k", p=P),
            outs_acc[:, ti0 : ti0 + n_sub, :],
        )

    # Software-pipelined emission: k-loops are emitted ahead of the previous
    # blocks' routing, so each engine's strict-FIFO queue never has a routing
    # instruction (whose deps resolve at block end) ahead of the next block's
    # casts/matmuls (head-of-line blocking). Blocks 0..PRO-1 run as a k-outer
    # prologue; their routings drain after block PRO's k-loop.
    PRO = 1
    assert n_blk > PRO
    psums_by_tb = {}
    for tb in range(PRO):
        psums_by_tb[tb] = [
            ppool.tile([P, 2, E], f32, name=f"psum_{tb}_{s}", tag="psum")
            for s in range(n_sub)
        ]
    emit_prologue(list(range(PRO)), psums_by_tb)
    pending = list((tb * BLK, psums_by_tb[tb]) for tb in range(PRO))
    for tb in range(PRO, n_blk):
        psums = [
            ppool.tile([P, 2, E], f32, name=f"psum_{tb}_{s}", tag="psum")
            for s in range(n_sub)
        ]
        emit_kloop(tb, psums, routing_pending=pending)
        for args in pending:
            emit_routing_dma(*args)
        pending = [(tb * BLK, psums)]
    emit_corr(*corr_lag[0])
    corr_lag[0] = None
    for args in pending:
        emit_routing(*args)
        emit_routing_dma(*args)


_NC_CACHE = {}


TAPER = False


def _build(mode=MODE, t_core=T_CORE, blk=BLK, repeat=1, taper=None):
    if taper is None:
        taper = TAPER
    key = (mode, t_core, blk, repeat, taper)
    if key in _NC_CACHE:
        return _NC_CACHE[key]
    nc = bacc.Bacc("TRN2", target_bir_lowering=False, debug=False)
    f32 = mybir.dt.float32
    f16 = mybir.dt.float16
    if mode == "f16dr":
        blk = DR_BLK
        n_blk = t_core // blk
        n_k = H // P
        ins = [
            nc.dram_tensor(
                "xh", [P, n_blk, n_k, blk], f16, kind="ExternalInput"
            ).ap(),
            nc.dram_tensor(
                "xl8",
                [P, n_blk, n_k, blk],
                mybir.dt.float8e4,
                kind="ExternalInput",
            ).ap(),
            nc.dram_tensor("w16", [H, E], f16, kind="ExternalInput").ap(),
            nc.dram_tensor(
                "dw8", [P, n_k, E], mybir.dt.float8e4, kind="ExternalInput"
            ).ap(),
            nc.dram_tensor("bias", [E], f32, kind="ExternalInput").ap(),
        ]
        outs = [
            nc.dram_tensor("wts", [t_core, K], f32, kind="ExternalOutput").ap(),
            nc.dram_tensor(
                "sel", [t_core, K], mybir.dt.int32, kind="ExternalOutput"
            ).ap(),
        ]
        with tile.TileContext(nc) as tc:
            _gate_kernel_f16dr(tc, outs, ins, T_core=t_core, BLK=blk, kpack=DR_KPACK)
        nc.compile()
        _NC_CACHE[key] = nc
        return nc
    if mode == "f16x3":
        ins = [
            nc.dram_tensor("xh", [H, t_core], f16, kind="ExternalInput").ap(),
            nc.dram_tensor("xl", [H, t_core], f16, kind="ExternalInput").ap(),
            nc.dram_tensor("wh", [H, E], f16, kind="ExternalInput").ap(),
            nc.dram_tensor("wl", [H, E], f16, kind="ExternalInput").ap(),
            nc.dram_tensor("bias", [E], f32, kind="ExternalInput").ap(),
        ]
    else:
        mdt = mybir.dt.float32r if mode == "f32r" else f32
        ins = [
            nc.dram_tensor("xT", [H, t_core], mdt, kind="ExternalInput").ap(),
            nc.dram_tensor("w", [H, E], mdt, kind="ExternalInput").ap(),
            nc.dram_tensor("bias", [E], f32, kind="ExternalInput").ap(),
        ]
    outs = [
        nc.dram_tensor("wts", [t_core, K], f32, kind="ExternalOutput").ap(),
        nc.dram_tensor("sel", [t_core, K], mybir.dt.int32, kind="ExternalOutput").ap(),
    ]
    with tile.TileContext(nc) as tc:
        _gate_kernel(
            tc, outs, ins, T_core=t_core, BLK=blk, mode=mode, repeat=repeat,
            taper=taper,
        )
    nc.compile()
    _NC_CACHE[key] = nc
    return nc


def _make_in_maps(x, W_gate, bias, mode=MODE):
    x = np.asarray(x, dtype=np.float32)
    W_gate = np.asarray(W_gate, dtype=np.float32)
    bias = np.asarray(bias, dtype=np.float32)
    in_maps = []
    if mode == "f16dr":
        e4 = ml_dtypes.float8_e4m3
        W16 = W_gate.astype(np.float16)
        dW = (W_gate - W16.astype(np.float32)).astype(np.float32)
        dw8 = (dW * np.float32(2.0**17)).astype(e4)
        # partition-major panel [P, n_k, E]: row h = k*128 + p
        dw8 = np.ascontiguousarray(
            dw8.reshape(H // 128, 128, E).transpose(1, 0, 2)
        )
        n_blk = T_CORE // DR_BLK
        n_k = H // 128
        for c in range(N_CORES):
            xT = x[c * T_CORE : (c + 1) * T_CORE].T  # [H, T_CORE]
            xh = xT.astype(np.float16)
            xl8 = ((xT - xh.astype(np.float32)) * np.float32(2.0**11)).astype(e4)
            # block-panel: [H, T] -> [P, n_blk, n_k, blk]
            # H = n_k*P with partition p at row k*P + p; token t = tb*blk + j
            xh_p = np.ascontiguousarray(
                xh.reshape(n_k, 128, n_blk, DR_BLK).transpose(1, 2, 0, 3)
            )
            xl8_p = np.ascontiguousarray(
                xl8.reshape(n_k, 128, n_blk, DR_BLK).transpose(1, 2, 0, 3)
            )
            in_maps.append(
                {"xh": xh_p, "xl8": xl8_p, "w16": W16, "dw8": dw8, "bias": bias}
            )
        return in_maps
    if mode == "f16x3":
        Wh = W_gate.astype(np.float16)
        Wl = (W_gate - Wh.astype(np.float32)).astype(np.float16)
        for c in range(N_CORES):
            xT = x[c * T_CORE : (c + 1) * T_CORE].T
            xh = np.ascontiguousarray(xT.astype(np.float16))
            xl = np.ascontiguousarray(
                (xT - xh.astype(np.float32)).astype(np.float16)
            )
            in_maps.append({"xh": xh, "xl": xl, "wh": Wh, "wl": Wl, "bias": bias})
    else:
        for c in range(N_CORES):
            xT = np.ascontiguousarray(x[c * T_CORE : (c + 1) * T_CORE].T)
            in_maps.append({"xT": xT, "w": W_gate, "bias": bias})
    return in_maps


_NEFF_CACHE_DIR = "/tmp/bass_neff_cache"
_neff_cache_installed = False


def _install_neff_cache():
    """Cache compiled NEFFs by BIR hash so repeat runs skip walrus."""
    global _neff_cache_installed
    if _neff_cache_installed:
        return
    import hashlib
    import os
    import shutil

    from concourse import bass2jax, bass_utils

    orig = bass_utils.compile_bir_kernel

    def cached(bir_json, tmpdir, neff_name="file.neff"):
        h = hashlib.sha256(bir_json).hexdigest()[:24]
        os.makedirs(_NEFF_CACHE_DIR, exist_ok=True)
        cpath = os.path.join(_NEFF_CACHE_DIR, h + ".neff")
        out = os.path.join(tmpdir, neff_name)
        if os.path.exists(cpath):
            shutil.copy(cpath, out)
            return out
        p = orig(bir_json, tmpdir, neff_name)
        try:
            shutil.copy(p, cpath)
        except OSError:
            pass
        return p

    bass2jax.compile_bir_kernel = cached
    _neff_cache_installed = True


def run_on_hw(x, W_gate, bias, mode=MODE, trace=False, **kwargs):
    from concourse import bass_utils

    _install_neff_cache()
    nc = _build(mode)
    in_maps = _make_in_maps(x, W_gate, bias, mode)
    res = bass_utils.run_bass_kernel_spmd(
        nc, in_maps, list(range(N_CORES)), trace=trace, **kwargs
    )
    wts = np.concatenate([r["wts"] for r in res.results], axis=0)
    sel = np.concatenate([r["sel"] for r in res.results], axis=0)
    return (wts.astype(np.float32), sel.astype(np.int32)), res


def kernel(x, W_gate, bias):
    (wts, sel), _ = run_on_hw(x, W_gate, bias, MODE)
    return wts, sel

